# revision 24
# baseline (speedup 1.0000x reference)
"""MinkowskiGlobalPooling (average=True) segment-mean kernel for 8 trn2 cores.

Full inputs in, full output out. Strategy (mask-stationary fp8 DoubleRow):
  - rows are sharded across 8 cores (500k rows each), laid out per core as
    128 SBUF partitions x R=3920 rows (tail rows of the last lane padded
    with zeros),
  - feats are quantized host-side to float8 e4m3 with SIGMA-DELTA error
    feedback along each lane's row chain (carry resets at batch
    boundaries), so the segment-SUM error collapses to ~one quantization
    step per (lane,segment,channel) chain instead of growing like
    sqrt(N): rel-err ~1e-3, far under the 2e-2 gate,
  - batch_idx is sorted, so each lane's batch id is piecewise-constant
    with at most ~7 changes per core. The matmul is therefore flipped
    vs. the classic formulation: the STATIONARY operand is the per-lane
    one-hot batch mask [128, 2, 8] (one version per chunk, from a tiny
    host-built table), and the MOVING operand is the feats stream
    [128, 2, 512] in fp8 DoubleRow mode (2 k-tiles, 2 cols/cycle):
    one matmul ingests 16 row-positions x 128 lanes = 2048 rows.
    psum[j, k*64+c] accumulates the k-th position-slot partial sums;
    the 8 slots are folded on the host,
  - rows whose true batch differs from their chunk's mask version (only
    the <=7 boundary tails per core) are re-attributed host-side using
    the exact quantized values, so the device result stays exact,
  - counts come from a host-side bincount (exact integers either way),
  - the whole stream rides ONE HWDGE queue (scalar) in chunks sized for
    fat DMA packets (384-row chunks = 24.5KB per partition-line packet,
    ~27GB/s per DMA engine x 16 engines ~ the 435GB/s SBUF fabric
    ceiling), with a deep 7-buffer ring so the engines never starve, a
    small lead-in chunk, and a tapered tail so the last chunks' matmuls
    finish right after their DMA,
  - host folds the per-core [8, 512] psums into the global [32, 64] and
    divides by counts.
"""

import numpy as np


def _ensure_import_path():
    try:
        import concourse.bass  # noqa: F401
    except ImportError:
        import sys

        for p in ("/opt/trn_rl_repo", "/root/.axon_site/_ro/trn_rl_repo"):
            if p not in sys.path:
                sys.path.insert(0, p)


N_CORES = 8
B = 32  # global batches
W = 8  # local batch window per core (sorted batch_idx => width <= 8)
C = 64  # channels
N_TOTAL = 4_000_000
N_CORE = N_TOTAL // N_CORES  # 500_000 real rows per core
P = 128  # SBUF partitions
R = 3920  # rows per partition (128*3920 = 501_760 >= 500_000; tail is padding)
G = 16  # row-positions per matmul (2 k-tiles x 8 slots)
K = 8  # position slots folded on host

# 384-row chunks = 24.5KB partition-line packets (~27GB/s per DMA engine);
# small lead-in so matmuls start early, tapered tail so the last chunks'
# matmuls finish right behind their DMA. 7-deep ring keeps engines fed.
SCHEDULE = [80] + [384] * 9 + [192, 112, 48, 32]
assert sum(SCHEDULE) == R, sum(SCHEDULE)
assert all(t % G == 0 for t in SCHEDULE)
N_CHUNKS = len(SCHEDULE)
N_GROUPS = R // G  # matmuls per core
FBUFS = 7  # feats chunk buffers
# Inter-chunk DRAM padding (bytes). Measured: 12KB pads, meant to rotate
# each DMA engine's HBM-channel window per chunk, instead lock ALL runs
# into the slow (~110us) contention mode — the SPMD-identical rotation
# keeps every core's engines synchronized on the same channel windows.
# The unpadded static layout is bimodal (94us lucky / 110us unlucky) and
# strictly dominates. Keep 0.
CHUNK_PAD = 0


def _byte_offs(schedule):
    bo = [0]
    for t in schedule:
        bo.append(bo[-1] + P * t * C + CHUNK_PAD)
    return bo


def build_program(schedule=None, fbufs=FBUFS):
    """Build the per-core Bass program. All cores run the identical program."""
    _ensure_import_path()
    import concourse.mybir as mybir
    from concourse import bacc
    from concourse.tile import TileContext

    f32 = mybir.dt.float32
    f8 = mybir.dt.float8e4
    if schedule is None:
        schedule = SCHEDULE
    n_chunks = len(schedule)
    n_groups = sum(schedule) // G
    half = n_groups // 2

    offs = [0]
    for t in schedule:
        offs.append(offs[-1] + t)

    boffs = _byte_offs(schedule)
    nc = bacc.Bacc()
    stream = nc.dram_tensor("stream", [boffs[-1]], f8, kind="ExternalInput")
    # per-chunk stationary masks, kt-duplicated and padded to a 16B k-tile
    # step (ISA: dual-fp8 ldweights requires kt step % 16 == 0):
    # [p, chunk, 2, 16] with cols 0..W-1 one-hot, rest zero
    masks = nc.dram_tensor("masks", [P * n_chunks * 32], f8, kind="ExternalInput")
    outa = nc.dram_tensor("outa", [W, K * C], f32, kind="ExternalOutput")
    outb = nc.dram_tensor("outb", [W, K * C], f32, kind="ExternalOutput")

    with TileContext(nc) as tc:
        with (
            tc.tile_pool(name="const", bufs=1) as cpool,
            tc.tile_pool(name="feats", bufs=fbufs) as fpool,
            tc.tile_pool(name="psum", bufs=1, space="PSUM") as ppool,
            tc.tile_pool(name="outp", bufs=1) as opool,
        ):
            mask_sb = cpool.tile([P, n_chunks * 32], f8)
            mask_dram = masks[:].rearrange("(p x) -> p x", p=P)
            # mask table rides the otherwise-idle gpsimd queue
            nc.gpsimd.dma_start(out=mask_sb[:], in_=mask_dram[:, :])

            psum_a = ppool.tile([W, K * C], f32)
            psum_b = ppool.tile([W, K * C], f32)
            psums = [psum_a, psum_b]

            g = 0  # global group index
            for ci, t in enumerate(schedule):
                ft = fpool.tile([P, t * C], f8, tag="ft")
                nc.scalar.dma_start(
                    out=ft[:],
                    in_=stream[boffs[ci] : boffs[ci] + P * t * C].rearrange(
                        "(p x) -> p x", p=P
                    ),
                )
                mk = mask_sb[:, ci * 32 : (ci + 1) * 32].rearrange(
                    "p (two j) -> p two j", two=2
                )[:, :, 0:W]
                for s in range(t // G):
                    h = 0 if g < half else 1
                    # each matmul writes the full [W, K*C] region, so a
                    # plain start on the group's first matmul is safe
                    nc.tensor.matmul(
                        psums[h][:],
                        lhsT=mk,
                        rhs=ft[:, s * G * C : (s + 1) * G * C].rearrange(
                            "p (two x) -> p two x", two=2
                        ),
                        start=(g in (0, half)),
                        stop=(g in (half - 1, n_groups - 1)),
                        perf_mode=mybir.MatmulPerfMode.DoubleRow,
                    )
                    g += 1
                    if g == half:
                        # first half's readout hides under the remaining
                        # stream; outputs ride the otherwise-idle sync ring
                        outa_sb = opool.tile([W, K * C], f32, tag="oa")
                        nc.vector.tensor_copy(out=outa_sb[:], in_=psums[0][:])
                        nc.sync.dma_start(out=outa[:, :], in_=outa_sb[:])
            outb_sb = opool.tile([W, K * C], f32, tag="ob")
            nc.vector.tensor_copy(out=outb_sb[:], in_=psums[1][:])
            nc.sync.dma_start(out=outb[:, :], in_=outb_sb[:])
    nc.finalize()
    return nc


def host_prep(feats, batch_idx):
    """Build per-core input maps (sigma-delta fp8 stream + mask table) from
    full inputs. Returns (in_maps, aux) where aux carries everything the
    finalize step needs (window offsets, counts, boundary corrections)."""
    import ml_dtypes

    f8 = ml_dtypes.float8_e4m3fn
    feats = np.asarray(feats, dtype=np.float32)
    bi = np.asarray(batch_idx).astype(np.int64)
    n, c = feats.shape
    assert n == N_TOTAL and c == C, (n, c)

    counts = np.bincount(bi, minlength=B).astype(np.float64)

    # ---- padded [cores*lanes, R, C] view with zero padding ----
    n_lanes = N_CORES * P
    xpad = np.zeros((n_lanes * R, C), dtype=np.float32)
    bpad = np.full(n_lanes * R, -1, dtype=np.int64)
    for m in range(N_CORES):
        sl = slice(m * N_CORE, (m + 1) * N_CORE)
        dst = slice(m * P * R, m * P * R + N_CORE)
        xpad[dst] = feats[sl]
        bpad[dst] = bi[sl]
    xpad = xpad.reshape(n_lanes, R, C)
    bpad = bpad.reshape(n_lanes, R)

    # ---- sigma-delta e4m3 quantization along each lane chain ----
    # carry resets where the batch changes (incl. entering padding)
    same = np.empty((n_lanes, R), dtype=bool)
    same[:, 0] = False
    same[:, 1:] = bpad[:, 1:] == bpad[:, :-1]
    q = np.empty((n_lanes, R, C), dtype=f8)
    carry = np.zeros((n_lanes, C), dtype=np.float32)
    for i in range(R):
        v = xpad[:, i, :] + carry * same[:, i, None]
        qi = v.astype(f8)
        q[:, i, :] = qi
        carry = v - qi.astype(np.float32)

    offs = np.concatenate([[0], np.cumsum(SCHEDULE)])

    in_maps = []
    lo_ws = []
    corr = np.zeros((B, C), dtype=np.float64)
    for m in range(N_CORES):
        qv = q[m * P : (m + 1) * P]  # [P, R, C]
        bv = bpad[m * P : (m + 1) * P]  # [P, R]
        real = bv >= 0
        lo = int(bi[m * N_CORE])
        hi = int(bi[(m + 1) * N_CORE - 1])
        assert hi - lo + 1 <= W, (m, lo, hi)
        lo_w = min(lo, B - W)
        lo_ws.append(lo_w)

        # per-chunk mask version: lane batch at chunk start (-1 if pad)
        j0 = bv[:, offs[:-1]]  # [P, n_chunks]
        # device attribution per position
        dev = np.repeat(j0, SCHEDULE, axis=1)  # [P, R]
        # boundary corrections: move quantized values dev->true
        mism = (dev != bv) & real
        if mism.any():
            pi, ri = np.nonzero(mism)
            qf = qv[pi, ri].astype(np.float64)  # [n, C]
            np.add.at(corr, bv[pi, ri], qf)
            np.subtract.at(corr, dev[pi, ri], qf)

        # mask table [P, n_chunks, 2, 16] one-hot of local j (kt-duplicated,
        # padded to 16B k-tile step for the dual-fp8 ldweights ISA rule)
        mt = np.zeros((P, N_CHUNKS, 2, 16), dtype=f8)
        jl = j0 - lo_w  # [P, n_chunks]; negative where pad
        valid = j0 >= 0
        pidx, cidx = np.nonzero(valid)
        mt[pidx, cidx, :, jl[pidx, cidx]] = 1.0

        # chunk-major flat stream with inter-chunk channel-rotation pads:
        # chunk ci = [p, t, C] contiguous block at _byte_offs()[ci]
        boffs = _byte_offs(SCHEDULE)
        flat = np.zeros(boffs[-1], dtype=f8)
        for ci, t in enumerate(SCHEDULE):
            blk = qv[:, offs[ci] : offs[ci] + t]  # [P, t, C]
            flat[boffs[ci] : boffs[ci] + blk.size] = blk.reshape(-1)
        in_maps.append({"stream": flat, "masks": mt.reshape(-1)})
    return in_maps, (lo_ws, counts, corr)


_CACHED_NC = None


def get_program():
    global _CACHED_NC
    if _CACHED_NC is None:
        _CACHED_NC = build_program()
    return _CACHED_NC


def run_on_cores(in_maps, trace=False):
    _ensure_import_path()
    from concourse.bass_utils import run_bass_kernel_spmd

    nc = get_program()
    res = run_bass_kernel_spmd(nc, in_maps, list(range(N_CORES)), trace=trace)
    return res


def finalize(per_core_outs, aux):
    lo_ws, counts, corr = aux
    sums = np.zeros((B, C), dtype=np.float64)
    for o, lo_w in zip(per_core_outs, lo_ws):
        o = np.asarray(o, dtype=np.float64).reshape(W, K, C)  # halves pre-added
        sums[lo_w : lo_w + W] += o.sum(axis=1)
    sums += corr
    pooled = sums / np.maximum(counts, 1.0)[:, None]
    return pooled.astype(np.float32)


def kernel(feats, batch_idx, num_batches):
    assert int(num_batches) == B
    in_maps, aux = host_prep(feats, batch_idx)
    res = run_on_cores(in_maps)
    outs = [
        np.asarray(r["outa"], dtype=np.float64) + np.asarray(r["outb"], np.float64)
        for r in res.results
    ]
    return finalize(outs, aux)
